# revision 44
# baseline (speedup 1.0000x reference)
"""Distributed Trainium2 kernel for a dense transformer block.

Sharding: sequence-parallel over the 8 NeuronCores. The flattened
[B*S=4096, D=1024] token stream is split into 8 contiguous shards of 512
tokens (cores 0-3 hold batch 0, cores 4-7 hold batch 1). Weights are
replicated; the only collectives are AllGathers of each core's K^T and V
within its 4-core batch group.

Structure:
 - Host pre-pass (untimed): weights cast to bf16 and laid out for direct
   DMA consumption; LN1 gamma/beta folded into Wqkv/bqkv, LN2 into
   W1/b1; the V bias folded through attention into bo (softmax rows sum
   to 1), so the device applies no LN affine and no V/O bias ops.
 - K is computed first and its AllGather issued immediately; V follows
   in a second AllGather that hides under the first attention pairs; Q
   overlaps the gathers.
 - Attention uses a transposed AV with DoubleRow fp8: V (plus a ones
   column that accumulates the softmax denominator) is the stationary
   operand [keys, 2, 65], probs stream as the moving operand, producing
   attnT [dh, tok] plus the denominator row directly in PSUM - no
   output transposes, no small-N matmuls.
 - The denominator reciprocal row is broadcast across partitions with a
   K=1 PE matmul (ones column x rec row); gpsimd partition_broadcast
   silently reads partition 0 on hardware, so it is only used for
   base-0 rows.
 - Software pipelining against the in-order engine queues: AV for jj
   runs one jj behind its exp; normalize for pair pr-1 slots into pair
   pr's j-loop, so no PE instruction waits on the slow DVE normalize
   chain (the [1,512] reciprocal is a ~3us single-partition op). Wo runs
   as a dense post-attention phase: 8 psum-chained slots accumulate over
   all head pairs back-to-back (PE stays warm), one residual add per
   slot instead of per-pair SBUF adds.
 - PSUM: mm pool 2x[128,2,512] = 4 banks, av pool 4x[128,512] = 4 banks
   (AV accumulators, V-phase/W2 extra chains, transpose staging).
"""

import sys

if "/opt/trn_rl_repo" not in sys.path:
    sys.path.insert(0, "/opt/trn_rl_repo")

import numpy as np

B, S, D = 2, 2048, 1024
H, DH, FF = 16, 64, 4096
NCORES = 8
TOK = (B * S) // NCORES      # 512 tokens per core
P = 128
TT = TOK // P                # 4 token tiles
KD = D // P                  # 8 contract tiles over D
FT = FF // P                 # 32 tiles over FF
GS = 4                       # group size (cores per batch)
NKJ = S // P                 # 16 key tiles per batch
GROUPS = [[0, 1, 2, 3], [4, 5, 6, 7]]
KELEMS = KD * P * TOK        # elements of one core's K^T / V (524288)
VCOLS = 66                   # v_aug row: 64 dh + ones + pad

_cache = {}
DEBUG = False


def _build():
    from contextlib import ExitStack
    from concourse import bacc, tile, mybir
    from concourse.masks import make_identity

    F32 = mybir.dt.float32
    BF16 = mybir.dt.bfloat16
    F8 = mybir.dt.float8e4
    Alu = mybir.AluOpType
    Act = mybir.ActivationFunctionType

    nc = bacc.Bacc("TRN2", target_bir_lowering=False, debug=False,
                   num_devices=NCORES)

    x_ext = nc.dram_tensor("x", [TOK, D], F32, kind="ExternalInput")
    wqkv_ext = nc.dram_tensor("wqkvT", [P, KD, 2 * D], BF16,
                              kind="ExternalInput")
    wv_ext = nc.dram_tensor("wv", [D, D], BF16, kind="ExternalInput")
    wo_ext = nc.dram_tensor("wo", [D, D], BF16, kind="ExternalInput")
    w1_ext = nc.dram_tensor("w1T", [P, KD, FF], BF16, kind="ExternalInput")
    w2_ext = nc.dram_tensor("w2", [FF, D], BF16, kind="ExternalInput")
    bqkv_ext = nc.dram_tensor("bqkv_qk", [P, 16], F32, kind="ExternalInput")
    b1_ext = nc.dram_tensor("b1col", [P, FT], F32, kind="ExternalInput")
    bo_ext = nc.dram_tensor("bo_eff", [D], F32, kind="ExternalInput")
    b2_ext = nc.dram_tensor("b2", [D], F32, kind="ExternalInput")
    out_ext = nc.dram_tensor("out", [TOK, D], F32, kind="ExternalOutput")
    if DEBUG:
        dbg_qt = nc.dram_tensor("dbg_qT", [P, KD, TOK], F8,
                                kind="ExternalOutput")
        dbg_kf = nc.dram_tensor("dbg_kf", [P, KD, GS, TOK], F8,
                                kind="ExternalOutput")
        dbg_va = nc.dram_tensor("dbg_va", [P, NKJ, H, VCOLS], F8,
                                kind="ExternalOutput")
        dbg_at = nc.dram_tensor("dbg_attnT", [P, KD, TOK], BF16,
                                kind="ExternalOutput")
        dbg_x1 = nc.dram_tensor("dbg_x1", [P, TT, D], F32,
                                kind="ExternalOutput")
        dbg_g1 = nc.dram_tensor("dbg_g1T", [P, FT, TOK], F8,
                                kind="ExternalOutput")

    with tile.TileContext(nc) as tc, ExitStack() as ctx:
        const = ctx.enter_context(tc.tile_pool(name="const", bufs=1))
        persist = ctx.enter_context(tc.tile_pool(name="persist", bufs=1))
        wcol = ctx.enter_context(tc.tile_pool(name="wcol", bufs=3))
        wchunk = ctx.enter_context(tc.tile_pool(name="wchunk", bufs=4))
        wopool = ctx.enter_context(tc.tile_pool(name="wopool", bufs=2))
        wvpool = ctx.enter_context(tc.tile_pool(name="wvpool", bufs=3))
        act = ctx.enter_context(tc.tile_pool(name="act", bufs=2))
        probsp = ctx.enter_context(tc.tile_pool(name="probsp", bufs=13))
        recp = ctx.enter_context(tc.tile_pool(name="recp", bufs=4))
        bcp = ctx.enter_context(tc.tile_pool(name="bcp", bufs=1))
        stgp = ctx.enter_context(tc.tile_pool(name="stgp", bufs=1))
        mm_ps = ctx.enter_context(
            tc.tile_pool(name="mm_ps", bufs=2, space="PSUM"))
        av_ps = ctx.enter_context(
            tc.tile_pool(name="av_ps", bufs=4, space="PSUM"))
        dram = ctx.enter_context(tc.tile_pool(name="dram", bufs=1, space="DRAM"))

        # x lands first so LN1 can start as early as possible; spread the
        # four tiles over different issue queues so they arrive in parallel
        x1_sb = persist.tile([P, TT, D], F32, tag="x1")
        xeng = [nc.sync, nc.scalar, nc.sync, nc.scalar]
        for t in range(TT):
            xeng[t].dma_start(x1_sb[:, t, :], x_ext[t * P:(t + 1) * P, :])

        # ---------------- constants ----------------
        eps_t = const.tile([P, 1], F32)
        nc.vector.memset(eps_t[:], 1e-5)
        ident = const.tile([P, P], BF16)
        make_identity(nc, ident[:])
        # ones column at partition 64 for the K=1 broadcast matmul (the
        # softmax denominator row lives at psum partition 64)
        ones_pb = const.tile([P, 64], BF16)
        nc.vector.memset(ones_pb[:], 1.0)
        bqkv_qk = const.tile([P, 16], F32)
        nc.sync.dma_start(bqkv_qk[:], bqkv_ext[:, :])
        b1col = const.tile([P, FT], F32)
        nc.sync.dma_start(b1col[:], b1_ext[:, :])

        # bias rows broadcast over partitions (gpsimd, off critical path)
        def bcast(src, name):
            row = act.tile([1, D], F32, tag="crow", name=f"{name}_row")
            nc.gpsimd.dma_start(row[:], src[:].rearrange("(a d) -> a d", a=1))
            full = const.tile([P, D], F32, name=f"{name}_bc")
            nc.gpsimd.partition_broadcast(full[:], row[:])
            return full

        bo_bc = bcast(bo_ext, "bo")
        b2_bc = bcast(b2_ext, "b2")

        # ---------------- helpers ----------------
        def layer_norm(x_ap, out_ap, eng=None):
            # stats stay on DVE (gpsimd has no bn_stats); the heavy xhat
            # tensor_scalar can run on gpsimd to spread the load
            eng = eng or nc.vector
            stats = act.tile([P, 2, 6], F32, tag="ln_stats", name="ln_stats")
            nc.vector.bn_stats(stats[:, 0, :], x_ap[:, 0:512])
            nc.vector.bn_stats(stats[:, 1, :], x_ap[:, 512:1024])
            mv = act.tile([P, 2], F32, tag="ln_mv", name="ln_mv")
            nc.vector.bn_aggr(mv[:], stats[:])
            rs = act.tile([P, 1], F32, tag="ln_rs", name="ln_rs")
            nc.scalar.activation(rs[:], mv[:, 1:2], Act.Sqrt, bias=eps_t[:])
            nc.vector.reciprocal(rs[:], rs[:])
            eng.tensor_scalar(out_ap, x_ap, scalar1=mv[:, 0:1],
                              scalar2=rs[:], op0=Alu.subtract,
                              op1=Alu.mult)

        def pe_transpose(dst_ap, src_ap):
            tp = av_ps.tile([P, P], BF16, tag="av", name="tp_ps")
            nc.tensor.transpose(tp[:], src_ap, ident[:])
            nc.vector.tensor_copy(dst_ap, tp[:])

        # ---------------- phase 1: LN1 + transpose ----------------
        hT = persist.tile([P, KD, TOK], BF16, tag="actT")
        for t in range(TT):
            ht = act.tile([P, D], BF16, tag="hmt", name="hmt")
            layer_norm(x1_sb[:, t, :], ht[:])
            for k in range(KD):
                pe_transpose(hT[:, k, t * P:(t + 1) * P],
                             ht[:, k * P:(k + 1) * P])

        # residual pre-bias: x1 += bo_eff (after LN1 consumed the tiles)
        for t in range(TT):
            nc.vector.tensor_add(x1_sb[:, t, :], x1_sb[:, t, :], bo_bc[:])

        # ---------------- phase 2: K, gather K, V, gather V, Q -------
        qT = persist.tile([P, KD, TOK], F8, tag="qT")
        kTl = persist.tile([P, KD, TOK], F8, tag="kTl")

        def qk_block(mp):
            wb = wcol.tile([P, KD, 2 * P], BF16, tag="wcol", name="wcol")
            nc.sync.dma_start(wb[:], wqkv_ext[:, :, mp * 2 * P:(mp + 1) * 2 * P])
            ps = mm_ps.tile([P, 2, TOK], F32, tag="mm2", name="mm_qkv")
            for hf in range(2):
                for k in range(KD):
                    nc.tensor.matmul(ps[:, hf, :],
                                     wb[:, k, hf * P:(hf + 1) * P],
                                     hT[:, k, :],
                                     start=(k == 0), stop=(k == KD - 1))
            for hf in range(2):
                m = 2 * mp + hf
                dst = qT if m < 8 else kTl
                nc.vector.tensor_scalar_add(dst[:, m % 8, :], ps[:, hf, :],
                                            scalar1=bqkv_qk[:, m:m + 1])

        for mp in range(4, 8):      # K first
            qk_block(mp)

        # bounce + AllGather of K (issued as soon as K is done)
        cc_in_k = dram.tile([KELEMS], F8)
        nc.scalar.dma_start(
            cc_in_k[:].rearrange("(k p t) -> p k t", k=KD, p=P), kTl[:])
        cc_out_k = dram.tile([GS * KELEMS], F8)
        nc.gpsimd.collective_compute(
            "AllGather", Alu.bypass, ins=[cc_in_k[:]], outs=[cc_out_k[:]],
            replica_groups=GROUPS)

        # V in natural layout: v = h @ Wv (bias folded into bo on host).
        # Single pass using all 8 psum banks; each hT block stays loaded
        # for both column halves so LDWEIGHTS amortizes over 2 matmuls.
        v_sb = persist.tile([P, TT, D], F8, tag="vsb")
        wv_tiles = []
        for k in range(KD):
            wvb = wvpool.tile([P, D], BF16, tag="wv", name="wv")
            nc.sync.dma_start(wvb[:], wv_ext[k * P:(k + 1) * P, :])
            wv_tiles.append(wvb)
        pss_mm = [mm_ps.tile([P, 2, 512], F32, tag="mm2", name="mm_v")
                  for _ in range(2)]
        pss_av = [av_ps.tile([P, 512], F32, tag="av", name="mm_va")
                  for _ in range(4)]

        def vslot(t, c):
            if t < 2:
                return pss_mm[t][:, c, :]
            return pss_av[2 * (t - 2) + c][:, :]

        for k in range(KD):
            for t in range(TT):
                for c in range(2):
                    nc.tensor.matmul(vslot(t, c),
                                     hT[:, k, t * P:(t + 1) * P],
                                     wv_tiles[k][:, c * 512:(c + 1) * 512],
                                     start=(k == 0), stop=(k == KD - 1))
        for t in range(TT):
            for c in range(2):
                nc.vector.tensor_copy(v_sb[:, t, c * 512:(c + 1) * 512],
                                      vslot(t, c))

        # bounce + AllGather of V
        cc_in_v = dram.tile([KELEMS], F8)
        nc.scalar.dma_start(
            cc_in_v[:].rearrange("(t p d) -> p t d", t=TT, p=P), v_sb[:])
        cc_out_v = dram.tile([GS * KELEMS], F8)
        nc.gpsimd.collective_compute(
            "AllGather", Alu.bypass, ins=[cc_in_v[:]], outs=[cc_out_v[:]],
            replica_groups=GROUPS)

        for mp in range(0, 4):      # Q overlaps the gathers
            qk_block(mp)

        # gathered K^T / V-augmented layouts
        kT_full = persist.tile([P, KD, GS, TOK], F8, tag="ktfull_g1T")
        for r in range(GS):
            base = r * KELEMS
            eng = nc.gpsimd if r % 2 == 0 else nc.scalar
            eng.dma_start(
                kT_full[:, :, r, :],
                cc_out_k[base:base + KELEMS].rearrange(
                    "(k p t) -> p k t", k=KD, p=P))
        v_aug = persist.tile([P, NKJ, H, VCOLS], F8, tag="vaug")
        nc.vector.memset(v_aug[:, :, :, 64:65], 1.0)
        for r in range(GS):
            for vt in range(TT):
                vbase = r * KELEMS + vt * P * D
                nc.gpsimd.dma_start(
                    v_aug[:, r * TT + vt, :, 0:64],
                    cc_out_v[vbase:vbase + P * D].rearrange(
                        "(p h f) -> p h f", p=P, h=H))

        # ------- phase 3: attention with fused output projection -------
        # AV is transposed: lhsT = v_aug slice [keys,65] (stationary), rhs =
        # probs [keys, 512 queries] -> psum [65, 512] = attnT + denominator.
        attnT = persist.tile([P, KD, TOK], BF16, tag="kt_attnT")

        def softmax_recips(avs, pr):
            # reciprocal of the denominator rows (psum partition 64); slow
            # single-partition DVE ops, issued a full pair ahead of use
            recs = []
            for hp in range(2):
                rec = recp.tile([P, TOK], BF16, tag="rec", name="rec")
                with nc.allow_low_precision(
                        reason="bf16 softmax denominator reciprocal"):
                    nc.vector.reciprocal(rec[64:65, :], avs[hp][64:65, :])
                recs.append(rec)
            return recs

        def normalize(pr, avs, recs):
            # broadcast rec across partitions via ones[1,64].T @ rec[1,512]
            bc_ps = mm_ps.tile([P, 2, TOK], F32, tag="mm2", name="bc_ps")
            for hp in range(2):
                nc.tensor.matmul(bc_ps[0:64, hp, :], ones_pb[64:65, :],
                                 recs[hp][64:65, :], start=True, stop=True)
            bc = bcp.tile([P, 2, TOK], BF16, tag="bc", name="bc")
            nc.vector.tensor_copy(bc[0:64, :, :], bc_ps[0:64, :, :])
            nc.vector.tensor_mul(attnT[0:64, pr, :],
                                 avs[0][0:64, :], bc[0:64, 0, :])
            stg = stgp.tile([P, TOK], BF16, tag="stg", name="stg")
            nc.vector.tensor_mul(stg[0:64, :], avs[1][0:64, :],
                                 bc[0:64, 1, :])
            nc.gpsimd.dma_start(attnT[64:128, pr, :], stg[0:64, :])

        DR = mybir.MatmulPerfMode.DoubleRow
        prev = None      # (pr, avs, recs) of the previous pair
        for pr in range(H // 2):
            avs = [av_ps.tile([P, TOK], F32, tag="av", name="av_acc")
                   for _ in range(2)]
            def av_mm(jj, pb):
                # DoubleRow AV: both key blocks of the jj pair contract in
                # one matmul (2 fp8 weights per PE cell)
                for hp in range(2):
                    h = 2 * pr + hp
                    nc.tensor.matmul(
                        avs[hp][0:65, :],
                        v_aug[:, 2 * jj:2 * jj + 2, h, 0:65],
                        pb[:, hp, :, :],
                        start=(jj == 0), stop=(jj == NKJ // 2 - 1),
                        perf_mode=DR)

            pending = None   # (jj, probs) whose AV is deferred one step
            for jj in range(NKJ // 2):
                probs = probsp.tile([P, 2, 2, TOK], F8, tag="probs",
                                    name="probs")
                for jh in range(2):
                    j = 2 * jj + jh
                    r, jl = divmod(j, TT)
                    sp = mm_ps.tile([P, 2, TOK], F32, tag="mm2",
                                    name="mm_sc")
                    for hp in range(2):
                        lo = hp * 64
                        nc.tensor.matmul(
                            sp[:, hp, :],
                            kT_full[lo:lo + 64, pr, r, jl * P:(jl + 1) * P],
                            qT[lo:lo + 64, pr, :], start=True, stop=True)
                    nc.scalar.activation(probs[:, :, jh, :], sp[:],
                                         Act.Exp, scale=0.125)
                # AV for the PREVIOUS jj: its exps are already done, so
                # the in-order PE stream never waits on the Scalar engine
                if pending is not None:
                    av_mm(*pending)
                pending = (jj, probs)
                # pipeline: normalize pair pr-1 at jj==2 - the link into
                # the DVE normalize chain gets a full pair of slack so
                # the in-order PE stream never waits on it
                if prev is not None and jj == 2:
                    normalize(prev[0], prev[1], prev[2])
            av_mm(*pending)
            prev = (pr, avs, softmax_recips(avs, pr))

        # ------- dense Wo phase: psum chains over all pairs -------
        # Query-tiles 0/1 chain over pairs 0..6 BEFORE the last pair's
        # normalize, so the PE absorbs the ~6us DVE reciprocal latency of
        # pair 7 with useful work instead of stalling at its bc matmul.
        def wo_load(pr):
            wob = wopool.tile([P, D], BF16, tag="wo", name="wo")
            nc.gpsimd.dma_start(wob[:], wo_ext[pr * P:(pr + 1) * P, :])
            return wob

        def wo_links(slot_of, qts, prs, last_pr):
            for pr in prs:
                wob = wo_load(pr)
                for qt in qts:
                    for c in range(2):
                        nc.tensor.matmul(slot_of(qt, c),
                                         attnT[:, pr, qt * P:(qt + 1) * P],
                                         wob[:, c * 512:(c + 1) * 512],
                                         start=(pr == 0),
                                         stop=(pr == last_pr))

        def wo_drain(slot_of, qts):
            for qt in qts:
                for c in range(2):
                    sl = x1_sb[:, qt, c * 512:(c + 1) * 512]
                    nc.vector.tensor_add(sl, sl, slot_of(qt, c))

        NP = H // 2
        wo_mm1 = mm_ps.tile([P, 2, 512], F32, tag="mm2", name="wo_mm1")
        wo_av1 = [av_ps.tile([P, 512], F32, tag="av", name="wo_av1")
                  for _ in range(2)]

        def slot01(qt, c):
            return wo_mm1[:, c, :] if qt == 0 else wo_av1[c][:, :]

        wo_links(slot01, (0, 1), range(NP - 1), NP - 1)
        normalize(prev[0], prev[1], prev[2])
        wo_links(slot01, (0, 1), [NP - 1], NP - 1)
        wo_drain(slot01, (0, 1))

        wo_mm2 = mm_ps.tile([P, 2, 512], F32, tag="mm2", name="wo_mm2")
        wo_av2 = [av_ps.tile([P, 512], F32, tag="av", name="wo_av2")
                  for _ in range(2)]

        def slot23(qt, c):
            return wo_mm2[:, c, :] if qt == 2 else wo_av2[c][:, :]

        wo_links(slot23, (2, 3), range(NP), NP - 1)
        wo_drain(slot23, (2, 3))

        if DEBUG:
            nc.scalar.dma_start(dbg_qt[:, :, :], qT[:])
            nc.scalar.dma_start(dbg_kf[:, :, :, :], kT_full[:])
            nc.scalar.dma_start(dbg_va[:, :, :, :], v_aug[:])
            nc.scalar.dma_start(dbg_at[:, :, :], attnT[:])
            nc.scalar.dma_start(dbg_x1[:, :, :], x1_sb[:])

        # ---------------- phase 4: LN2 + transpose ----------------
        mT = persist.tile([P, KD, TOK], BF16, tag="actT2")
        for t in range(TT):
            mt = act.tile([P, D], BF16, tag="hmt", name="mlnt")
            layer_norm(x1_sb[:, t, :], mt[:])
            for k in range(KD):
                pe_transpose(mT[:, k, t * P:(t + 1) * P],
                             mt[:, k * P:(k + 1) * P])
        # final-residual pre-bias: x1 += b2 (after LN2 consumed the tiles)
        for t in range(TT):
            nc.vector.tensor_add(x1_sb[:, t, :], x1_sb[:, t, :], b2_bc[:])

        # ---------------- phase 5: MLP ----------------
        g1T = persist.tile([P, FT, TOK], BF16, tag="g1T")
        for mp in range(FT // 2):
            wb = wcol.tile([P, KD, 2 * P], BF16, tag="wcol", name="w1b")
            nc.sync.dma_start(wb[:], w1_ext[:, :, mp * 2 * P:(mp + 1) * 2 * P])
            ps = mm_ps.tile([P, 2, TOK], F32, tag="mm2", name="mm_w1")
            for hf in range(2):
                for k in range(KD):
                    nc.tensor.matmul(ps[:, hf, :],
                                     wb[:, k, hf * P:(hf + 1) * P],
                                     mT[:, k, :],
                                     start=(k == 0), stop=(k == KD - 1))
            for hf in range(2):
                m = 2 * mp + hf
                nc.scalar.activation(g1T[:, m, :], ps[:, hf, :],
                                     Act.Gelu_apprx_tanh,
                                     bias=b1col[:, m:m + 1])

        if DEBUG:
            nc.scalar.dma_start(dbg_g1[:, :, :], g1T[:])

        # W2: single pass over ff, 8 psum banks (2 mm tiles + 2 av tiles)
        pss2 = [mm_ps.tile([P, 2, 512], F32, tag="mm2", name="mm_w2")
                for _ in range(2)]
        pssa = [av_ps.tile([P, 512], F32, tag="av", name="mm_w2a")
                for _ in range(4)]

        def w2ps(c, qt):
            # c=0 -> the two pss2 tiles; c=1 -> the four av tiles
            if c == 0:
                return pss2[qt // 2][:, qt % 2, :]
            return pssa[qt][:, :]

        for ff in range(FT):
            w2b = wchunk.tile([P, D], BF16, tag="wch", name="w2b")
            nc.sync.dma_start(w2b[:], w2_ext[ff * P:(ff + 1) * P, :])
            for qt in range(TT):
                for c in range(2):
                    nc.tensor.matmul(w2ps(c, qt),
                                     g1T[:, ff, qt * P:(qt + 1) * P],
                                     w2b[:, c * 512:(c + 1) * 512],
                                     start=(ff == 0), stop=(ff == FT - 1))
        for qt in range(TT):
            for c in range(2):
                ot = act.tile([P, 512], F32, tag="oout", name="oout")
                nc.vector.tensor_add(ot[:], w2ps(c, qt),
                                     x1_sb[:, qt, c * 512:(c + 1) * 512])
                nc.sync.dma_start(
                    out_ext[qt * P:(qt + 1) * P, c * 512:(c + 1) * 512],
                    ot[:])

    nc.compile()
    return nc


def _get_nc():
    if "nc" not in _cache:
        _cache["nc"] = _build()
    return _cache["nc"]


def _prep_inputs(inputs):
    """Host-side weight prep: bf16 casts, LN folding, layout transforms."""
    import ml_dtypes
    bf16 = ml_dtypes.bfloat16
    f = {k: np.asarray(v, dtype=np.float32) for k, v in inputs.items()}

    wqkv = f["Wqkv"] * f["ln1_g"][:, None]
    bqkv_eff = f["bqkv"] + f["ln1_b"] @ f["Wqkv"]
    w1 = f["W1"] * f["ln2_g"][:, None]
    b1_eff = f["b1"] + f["ln2_b"] @ f["W1"]
    bo_eff = f["bo"] + bqkv_eff[2 * D:] @ f["Wo"]

    wqkvT = np.ascontiguousarray(
        wqkv[:, :2 * D].reshape(KD, P, 2 * D).transpose(1, 0, 2)
    ).astype(bf16)
    wv = np.ascontiguousarray(wqkv[:, 2 * D:]).astype(bf16)
    wo = np.ascontiguousarray(f["Wo"]).astype(bf16)
    w1T = np.ascontiguousarray(
        w1.reshape(KD, P, FF).transpose(1, 0, 2)).astype(bf16)
    w2 = np.ascontiguousarray(f["W2"]).astype(bf16)
    bqkv_qk = np.ascontiguousarray(
        bqkv_eff[:2 * D].reshape(16, P).T).astype(np.float32)
    b1col = np.ascontiguousarray(
        b1_eff.reshape(FT, P).T).astype(np.float32)

    weights = {
        "wqkvT": wqkvT, "wv": wv, "wo": wo, "w1T": w1T, "w2": w2,
        "bqkv_qk": bqkv_qk, "b1col": b1col,
        "bo_eff": bo_eff.astype(np.float32),
        "b2": f["b2"].astype(np.float32),
    }
    flat = np.ascontiguousarray(f["x"].reshape(B * S, D))
    in_maps = []
    for c in range(NCORES):
        m = {"x": np.ascontiguousarray(flat[c * TOK:(c + 1) * TOK])}
        m.update(weights)
        in_maps.append(m)
    return in_maps


def kernel(**inputs):
    from concourse.bass_utils import run_bass_kernel_spmd

    nc = _get_nc()
    in_maps = _prep_inputs(inputs)
    res = run_bass_kernel_spmd(nc, in_maps, core_ids=list(range(NCORES)))
    out = np.concatenate([res.results[c]["out"] for c in range(NCORES)],
                         axis=0)
    return out.reshape(B, S, D).astype(np.float32)
